# revision 22
# baseline (speedup 1.0000x reference)
"""Two-branch attention kernel for Trainium2 (8 NeuronCores, batch-parallel).

out1 = proj(softmax(q k^T / 8) v),  out2 = proj(softmax(q k2^T / 8) v2)
with q,k,v from x and k2,v2 from x2 (q shared across branches).

Sharding: batch dim (8) -> one batch element per core. No collectives.

v2 design (pipelined, fp16 operands):
  All matmul operands fp16 (PE rate identical to bf16/f32r@512, better
  numerics than bf16, halves SBUF so no DRAM spill is needed).
  Phase A:  q,k,k2 via W-stationary matmuls (k/k2 share weight tiles),
            v via x-stationary; evictions on ACT (q/k/k2) + DVE (v).
  Phase B1: branch-1 attention, per head h: S(kj)->exp(kj)->AV(kj)
            software pipeline (S psum double-buffered, po accumulates
            o^T plus row-sum via the ones-column trick). PE slack is
            filled with v2-formation thunks.
  Phase B2: branch-2 attention; PE slack filled with the branch-1
            1/r chain (PE transposes + one batched 128-lane DVE
            reciprocal), normalize muls, and proj(branch-1) thunks.
  Tail:     branch-2 1/r chain + normalize + proj(branch-2).
  1/r: row-sums r [12,1024] are PE-transposed to [128,96] so the DVE
       iterative-divide reciprocal (8 cyc/elem/lane) runs across 128
       lanes (0.8us) instead of 12 (8.5us); transposed back on PE and
       broadcast to 64 partitions via DRAM-bounce DMA.
"""
import sys
for _p in ('/opt/trn_rl_repo',):
    if _p not in sys.path:
        sys.path.insert(0, _p)

import numpy as np

MODE = 'fp16-pipelined-v2'

B, N, D, H, HD = 8, 1024, 768, 12, 64
SCALE = HD ** -0.5
NDT = D // 128       # 6 dim tiles
NQT = N // 128       # 8 token tiles
P = 128
AUG = HD + 1         # 65: head dim + ones column for row sums
NC = N // 512        # moving-dim chunks


# ----------------------------------------------------------------------------
# workaround: walrus rejects >2 sem waits on one instruction; TileContext's
# tail drain carries one wait per active logical proc. Split them across
# single-wait SP nops and emit a bare drain.
def _install_tilefix():
    import bass_rust
    import concourse.tile as tile

    def _drain_and_barrier_split(self, tick_clock, wait_clock):
        gc = tick_clock.global_clock
        ticks = [gc[i] for i in range(27)]
        for i, t in enumerate(ticks):
            if t > 0:
                vc = bass_rust.VectorClock(
                    [t if j == i else 0 for j in range(len(ticks))])
                nop = self.nc.sync.nop()
                wait_clock.add_sem_waits(
                    nop.ins, bass_rust.ScopedClock({None: vc}))
        self.nc.sync.drain()
        self.nc.all_engine_barrier()
        assert self.sems is not None
        popped = self.nc._tile_sem_poison_stack.pop()
        assert popped is self._sem_poison
        self.nc.clear_and_free_semaphores(list(self.sems.allocated().values()))
        self.nc.all_engine_barrier()

    tile.TileContext._drain_and_barrier = _drain_and_barrier_split


def _split_multiwaits(nc, max_waits=1):
    """walrus codegen rejects instructions carrying more than `max_waits`
    sync waits; hoist the extras onto same-engine nops placed just before."""
    import bass_rust
    import concourse.mybir as mybir
    cnt = 0
    for bb in nc.main_func.blocks:
        insts = bb.instructions
        i = 0
        while i < len(insts):
            ins = insts[i]
            si = getattr(ins, 'sync_info', None)
            if si is not None and si.on_wait and len(si.on_wait) > max_waits:
                waits = list(si.on_wait)
                extras, keep = waits[:-max_waits], waits[-max_waits:]
                for w in extras:
                    nop = mybir.InstNoOp(name=f"I-swx{cnt}", ins=[], outs=[])
                    cnt += 1
                    nop.engine = ins.engine
                    nop.sync_info = bass_rust.SyncInfo(on_wait=[w],
                                                       on_update=[])
                    insts.insert(i, nop)
                    i += 1
                ins.sync_info = bass_rust.SyncInfo(
                    on_wait=keep, on_update=list(si.on_update))
            i += 1
    return cnt


_built = None


def _build():
    """Build the SPMD bass program once. Returns (nc, n_split_waits)."""
    global _built
    if _built is not None:
        return _built
    _install_tilefix()
    from contextlib import ExitStack
    import concourse.bass as bass
    import concourse.tile as tile
    from concourse import mybir

    dt = mybir.dt
    ddt = dt.float16           # qkv/S/AV matmul operands
    rdt = dt.float32r          # proj matmul operands (o^T, W_p)
    f32 = dt.float32

    nc = bass.Bass("TRN2", target_bir_lowering=False, debug=False,
                   num_devices=8)

    # DRAM I/O (per core)
    xt_d = nc.dram_tensor("xt", [D, N], ddt, kind="ExternalInput")
    x2t_d = nc.dram_tensor("x2t", [D, N], ddt, kind="ExternalInput")
    wqk_d = nc.dram_tensor("wqk", [D, 2 * D], ddt, kind="ExternalInput")
    wv_d = nc.dram_tensor("wv", [D, D], ddt, kind="ExternalInput")
    wp_d = nc.dram_tensor("wp", [D, D], ddt, kind="ExternalInput")
    bias_d = nc.dram_tensor("bias", [P, D], f32, kind="ExternalInput")
    ones_d = nc.dram_tensor("ones", [P, NQT * H, 1], ddt,
                            kind="ExternalInput")
    id12_d = nc.dram_tensor("id12", [H, H], f32, kind="ExternalInput")
    id128_d = nc.dram_tensor("id128", [P, P], ddt, kind="ExternalInput")
    out_d = nc.dram_tensor("out", [2, N, D], f32, kind="ExternalOutput")

    with tile.TileContext(nc) as tc, ExitStack() as top:
        # PSUM pools: S double-buffered (4 banks) + po (2) + filler (2)
        pp_s = top.enter_context(tc.tile_pool(name="ps_s", bufs=2,
                                              space="PSUM"))
        pp_o = top.enter_context(tc.tile_pool(name="ps_o", bufs=1,
                                              space="PSUM"))
        pp_f = top.enter_context(tc.tile_pool(name="ps_f", bufs=1,
                                              space="PSUM"))
        dram = top.enter_context(tc.tile_pool(name="dram", bufs=1,
                                              space="DRAM"))
        persist = top.enter_context(tc.tile_pool(name="persist", bufs=1))
        pool_pt = top.enter_context(tc.tile_pool(name="pt", bufs=4))
        pool_rb = top.enter_context(tc.tile_pool(name="rb", bufs=4))
        pool_res = top.enter_context(tc.tile_pool(name="res", bufs=1))
        pool_rt = top.enter_context(tc.tile_pool(name="rt", bufs=2))

        # persistent SBUF
        qT = persist.tile([P, NDT, N], ddt, tag="qT")
        kT = persist.tile([P, NDT, N], ddt, tag="kT")
        kT2 = persist.tile([P, NDT, N], ddt, tag="kT2")
        vaug = persist.tile([P, NQT, H * AUG], ddt, tag="vaug")
        vaug2 = persist.tile([P, NQT, H * AUG], ddt, tag="vaug2")
        wp_t = persist.tile([P, NDT, D], ddt, tag="wp")
        bias_t = persist.tile([P, D], f32, tag="bias")
        id12_t = persist.tile([H, H], f32, tag="id12")
        id128_t = persist.tile([P, P], ddt, tag="id128")
        ot = [persist.tile([P, NDT, N], ddt, tag=f"ot{b}", name=f"ot{b}")
              for b in range(2)]
        r_all = [persist.tile([H, N], f32, tag=f"r{b}", name=f"r{b}")
                 for b in range(2)]
        rinvsw = [persist.tile([P, NQT * H], ddt, tag=f"risw{b}",
                               name=f"risw{b}") for b in range(2)]
        rinv = [persist.tile([H, N], ddt, tag=f"rinv{b}", name=f"rinv{b}")
                for b in range(2)]
        rinv_dram = [dram.tile([H, N], ddt, tag=f"rid{b}", name=f"rid{b}")
                     for b in range(2)]
        ones128 = persist.tile([P, N], f32, tag="ones128")
        nc.vector.memset(ones128[:], 1.0)

        nc.sync.dma_start(out=wp_t,
                          in_=wp_d[:].rearrange("(g p) d -> p g d", p=P))
        nc.sync.dma_start(out=bias_t, in_=bias_d[:])
        nc.sync.dma_start(out=id12_t, in_=id12_d[:])
        nc.sync.dma_start(out=id128_t, in_=id128_d[:])

        def load_ones(vaug_t):
            for t in range(NQT):
                nc.sync.dma_start(
                    out=vaug_t[:, t, :].rearrange("p (h e) -> p h e",
                                                  e=AUG)[:, :, HD:AUG],
                    in_=ones_d[:, t * H:(t + 1) * H, :])

        # ================= phase A: q, k, k2, v =========================
        pool_x = top.enter_context(tc.tile_pool(name="xa", bufs=1))
        pool_x2 = top.enter_context(tc.tile_pool(name="x2a", bufs=1))
        pool_wst = top.enter_context(tc.tile_pool(name="wst", bufs=4))
        pool_wv = top.enter_context(tc.tile_pool(name="wvp", bufs=1))

        xt_t = pool_x.tile([P, NDT, N], ddt, tag="xt")
        for i in range(NDT):
            nc.sync.dma_start(out=xt_t[:, i, :],
                              in_=xt_d[i * P:(i + 1) * P, :])
        x2t_t = pool_x2.tile([P, NDT, N], ddt, tag="x2t")
        wv_t = pool_wv.tile([P, NDT, D], ddt, tag="wv")

        def qT_form(o, wcol0, xt_s, dst, dst2):
            """one output-row-block o of W^T x (and W^T x2 if dst2)."""
            wt = pool_wst.tile([P, NDT, P], ddt, tag="wt")
            nc.sync.dma_start(
                out=wt,
                in_=wqk_d[:, wcol0 + o * P: wcol0 + (o + 1) * P].rearrange(
                    "(i p) o -> p i o", p=P))
            for xs, ds in ((xt_s, dst), (x2t_t, dst2)):
                if ds is None:
                    continue
                ps = pp_s.tile([P, N], f32, tag="S")
                for i in range(NDT):
                    for c in range(NC):
                        nc.tensor.matmul(
                            ps[:, c * 512:(c + 1) * 512],
                            wt[:, i, :],
                            xs[:, i, c * 512:(c + 1) * 512],
                            start=(i == 0), stop=(i == NDT - 1),
                            skip_group_check=True)
                nc.scalar.copy(ds[:, o, :], ps[:])

        def v_tile(t, xt_s, vaug_t):
            """one token-block t of v = x W_v^T, written into vaug."""
            ps = pp_f.tile([P, D], f32, tag="F")
            for i in range(NDT):
                for c0, cn in ((0, 512), (512, 256)):
                    nc.tensor.matmul(
                        ps[:, c0:c0 + cn],
                        xt_s[:, i, t * P:(t + 1) * P],
                        wv_t[:, i, c0:c0 + cn],
                        start=(i == 0), stop=(i == NDT - 1),
                        skip_group_check=True)
            src = ps[:, 0:D].rearrange("p (h e) -> p h e", e=HD)
            dstv = vaug_t[:, t, :].rearrange("p (h e) -> p h e",
                                             e=AUG)[:, :, 0:HD]
            nc.vector.tensor_copy(dstv, src)

        for o in range(NDT):
            qT_form(o, 0, xt_t, qT, None)            # q
        nc.sync.dma_start(out=x2t_t,
                          in_=x2t_d[:].rearrange("(i p) n -> p i n", p=P))
        nc.sync.dma_start(out=wv_t,
                          in_=wv_d[:].rearrange("(i p) d -> p i d", p=P))
        load_ones(vaug)
        for o in range(NDT):
            qT_form(o, D, xt_t, kT, kT2)             # k and k2 (shared W)
        load_ones(vaug2)
        for t in range(NQT):
            v_tile(t, xt_t, vaug)                    # v

        # ================= r-chain helpers ==============================
        def r_chain_front(b):
            """transpose r [12,N] -> [128,96], batched reciprocal,
            transpose back, copy to SBUF, bounce to DRAM. Returns list of
            PE thunks (transposes) + trailing ops emitted inline."""
            tp = pp_f.tile([P, NQT * H], f32, tag="F")
            for c in range(NQT):
                nc.tensor.matmul(
                    tp[:, c * H:(c + 1) * H],
                    r_all[b][:, c * P:(c + 1) * P],
                    id12_t[:],
                    is_transpose=True, skip_group_check=True)
            with nc.allow_low_precision(reason='1/r fp16'):
                nc.vector.reciprocal(rinvsw[b][:], tp[:])
            bp = pp_f.tile([H, N], ddt, tag="F")
            for c in range(NQT):
                nc.tensor.matmul(
                    bp[:, c * P:(c + 1) * P],
                    rinvsw[b][:, c * H:(c + 1) * H],
                    id128_t[:],
                    is_transpose=True, skip_group_check=True)
            nc.vector.tensor_copy(rinv[b][:], bp[:])
            nc.sync.dma_start(out=rinv_dram[b][:], in_=rinv[b][:])

        def norm_mul(b, h):
            # both TT inputs must share a start partition (walrus
            # samePartitionsAll); slice rb at the same offset as ot
            g, row = h // 2, (h % 2) * HD
            rb = pool_rb.tile([P, N], ddt, tag="rb")
            nc.sync.dma_start(
                out=rb[row:row + HD, :],
                in_=rinv_dram[b][h, :].partition_broadcast(HD))
            nc.vector.tensor_mul(ot[b][row:row + HD, g, :],
                                 ot[b][row:row + HD, g, :],
                                 rb[row:row + HD, :])

        def proj_qi(b, qi):
            ps = pp_f.tile([P, D], f32, tag="F")
            for g in range(NDT):
                for c0, cn in ((0, 512), (512, 256)):
                    nc.tensor.matmul(
                        ps[:, c0:c0 + cn],
                        ot[b][:, g, qi * P:(qi + 1) * P],
                        wp_t[:, g, c0:c0 + cn],
                        start=(g == 0), stop=(g == NDT - 1),
                        skip_group_check=True)
            res = pool_res.tile([P, D], f32, tag="res")
            nc.vector.tensor_add(res[:], ps[:], bias_t[:])
            nc.sync.dma_start(out=out_d[b, qi * P:(qi + 1) * P, :],
                              in_=res[:])

        # ================= phase B: attention ===========================
        def attention(b, kT_t, vaug_t, fillers):
            """per-head S->exp->AV pipeline; fillers is a list of
            (priority-ordered) thunks emitted into PE slack, ~1/head."""
            for h in range(H):
                g, row = h // 2, (h % 2) * HD
                po = pp_o.tile([AUG, N], f32, tag="po")
                pts = []

                def emit_S_exp(kj):
                    ps = pp_s.tile([P, N], f32, tag="S")
                    for c in range(NC):
                        nc.tensor.matmul(
                            ps[:, c * 512:(c + 1) * 512],
                            kT_t[row:row + HD, g, kj * P:(kj + 1) * P],
                            qT[row:row + HD, g, c * 512:(c + 1) * 512],
                            start=True, stop=True, skip_group_check=True)
                    pt = pool_pt.tile([P, N], ddt, tag="pt")
                    nc.scalar.activation(
                        pt[:], ps[:],
                        mybir.ActivationFunctionType.Exp, scale=SCALE)
                    pts.append(pt)

                def emit_av(kj):
                    for c in range(NC):
                        nc.tensor.matmul(
                            po[:, c * 512:(c + 1) * 512],
                            vaug_t[:, kj, h * AUG:(h + 1) * AUG],
                            pts[kj][:, c * 512:(c + 1) * 512],
                            start=(kj == 0), stop=(kj == NQT - 1),
                            skip_group_check=True)

                emit_S_exp(0)
                emit_S_exp(1)
                for kj in range(2, NQT):
                    emit_S_exp(kj)
                    emit_av(kj - 2)
                    if kj == 4 and fillers:
                        fillers.pop(0)()
                emit_av(NQT - 2)
                emit_av(NQT - 1)
                # evict o^T (unnormalized) and the row-sum r; TT-mul by
                # ones (not copy): walrus allows partition-offset APs on
                # TENSOR_TENSOR in f32 but not on 16-bit/copy forms
                nc.vector.tensor_mul(ot[b][row:row + HD, g, :], po[0:HD, :],
                                     ones128[0:HD, :])
                rtmp = pool_rt.tile([1, N], f32, tag="rtmp")
                nc.vector.tensor_mul(rtmp[0:1, :], po[HD:HD + 1, :],
                                     ones128[HD:HD + 1, :])
                # engines cannot write partition h directly; DMA-scatter
                nc.sync.dma_start(out=r_all[b][h:h + 1, :], in_=rtmp[0:1, :])

        # B1: branch-1 attention + v2-formation fillers
        fillers1 = [(lambda t=t: v_tile(t, x2t_t, vaug2)) for t in range(NQT)]
        attention(0, kT, vaug, fillers1)
        while fillers1:
            fillers1.pop(0)()

        # B2: branch-2 attention + (r-chain b0, norm b0, proj b0) fillers
        fillers2 = [lambda: r_chain_front(0)]
        fillers2 += [(lambda h=h: (norm_mul(0, 2 * h), norm_mul(0, 2 * h + 1)))
                     for h in range(H // 2)]
        fillers2 += [(lambda qi=qi: proj_qi(0, qi)) for qi in range(NQT)]
        attention(1, kT2, vaug2, fillers2)
        while fillers2:
            fillers2.pop(0)()

        # tail: branch-2 r-chain, normalize, proj
        r_chain_front(1)
        for h in range(H):
            norm_mul(1, h)
        for qi in range(NQT):
            proj_qi(1, qi)

    n = _split_multiwaits(nc)
    _built = (nc, n)
    return _built


def _host_prep(x, x2, qkv_w, proj_w, proj_b):
    """-> list of 8 per-core input maps. All matmul operands fp16."""
    f16 = lambda a: np.ascontiguousarray(a, dtype=np.float16)

    xt = np.transpose(np.asarray(x), (0, 2, 1))
    x2t = np.transpose(np.asarray(x2), (0, 2, 1))
    wqk = f16(np.asarray(qkv_w)[:2 * D].T)      # [768, 1536]
    wv = f16(np.asarray(qkv_w)[2 * D:].T)       # [768, 768]
    wp = f16(np.asarray(proj_w).T)              # [768, 768]
    bias = np.broadcast_to(np.asarray(proj_b, dtype=np.float32),
                           (P, D)).copy()
    ones = np.ones((P, NQT * H, 1), dtype=np.float16)
    id12 = np.eye(H, dtype=np.float32)
    id128 = np.eye(P, dtype=np.float16)
    maps = []
    for c in range(B):
        maps.append({
            "xt": f16(xt[c]), "x2t": f16(x2t[c]),
            "wqk": wqk, "wv": wv, "wp": wp, "bias": bias,
            "ones": ones, "id12": id12, "id128": id128,
        })
    return maps


def kernel(x, x2, qkv_w, proj_w, proj_b, trace=False, tmpdir=None):
    nc, _ = _build()
    from concourse.bass_utils import run_bass_kernel_spmd
    in_maps = _host_prep(x, x2, qkv_w, proj_w, proj_b)
    res = run_bass_kernel_spmd(nc, in_maps, list(range(B)), trace=trace,
                               tmpdir=tmpdir)
    kernel.last_exec_time_ns = res.exec_time_ns
    out = np.stack([res.results[c]["out"] for c in range(B)])  # [B,2,N,D]
    out1 = np.ascontiguousarray(out[:, 0])
    out2 = np.ascontiguousarray(out[:, 1])
    return (out1, out2)


kernel.last_exec_time_ns = None


# revision 23
# speedup vs baseline: 1.2587x; 1.2587x over previous
"""Two-branch attention kernel for Trainium2 (8 NeuronCores, batch-parallel).

out1 = proj(softmax(q k^T / 8) v),  out2 = proj(softmax(q k2^T / 8) v2)
with q,k,v from x and k2,v2 from x2 (q shared across branches).

Sharding: batch dim (8) -> one batch element per core. No collectives.

v2 design (pipelined, fp16 operands):
  All matmul operands fp16 (PE rate identical to bf16/f32r@512, better
  numerics than bf16, halves SBUF so no DRAM spill is needed).
  Phase A:  q,k,k2 via W-stationary matmuls (k/k2 share weight tiles),
            v via x-stationary; evictions on ACT (q/k/k2) + DVE (v).
  Phase B1: branch-1 attention, per head h: S(kj)->exp(kj)->AV(kj)
            software pipeline (S psum double-buffered, po accumulates
            o^T plus row-sum via the ones-column trick). PE slack is
            filled with v2-formation thunks.
  Phase B2: branch-2 attention; PE slack filled with the branch-1
            1/r chain (PE transposes + one batched 128-lane DVE
            reciprocal), normalize muls, and proj(branch-1) thunks.
  Tail:     branch-2 1/r chain + normalize + proj(branch-2).
  1/r: row-sums r [12,1024] are PE-transposed to [128,96] so the DVE
       iterative-divide reciprocal (8 cyc/elem/lane) runs across 128
       lanes (0.8us) instead of 12 (8.5us); transposed back on PE and
       broadcast to 64 partitions via DRAM-bounce DMA.
"""
import sys
for _p in ('/opt/trn_rl_repo',):
    if _p not in sys.path:
        sys.path.insert(0, _p)

import numpy as np

MODE = 'fp16-pipelined-v2'

B, N, D, H, HD = 8, 1024, 768, 12, 64
SCALE = HD ** -0.5
NDT = D // 128       # 6 dim tiles
NQT = N // 128       # 8 token tiles
P = 128
AUG = HD + 1         # 65: head dim + ones column for row sums
NC = N // 512        # moving-dim chunks


# ----------------------------------------------------------------------------
# workaround: walrus rejects >2 sem waits on one instruction; TileContext's
# tail drain carries one wait per active logical proc. Split them across
# single-wait SP nops and emit a bare drain.
def _install_tilefix():
    import bass_rust
    import concourse.tile as tile

    def _drain_and_barrier_split(self, tick_clock, wait_clock):
        gc = tick_clock.global_clock
        ticks = [gc[i] for i in range(27)]
        for i, t in enumerate(ticks):
            if t > 0:
                vc = bass_rust.VectorClock(
                    [t if j == i else 0 for j in range(len(ticks))])
                nop = self.nc.sync.nop()
                wait_clock.add_sem_waits(
                    nop.ins, bass_rust.ScopedClock({None: vc}))
        self.nc.sync.drain()
        self.nc.all_engine_barrier()
        assert self.sems is not None
        popped = self.nc._tile_sem_poison_stack.pop()
        assert popped is self._sem_poison
        self.nc.clear_and_free_semaphores(list(self.sems.allocated().values()))
        self.nc.all_engine_barrier()

    tile.TileContext._drain_and_barrier = _drain_and_barrier_split


def _split_multiwaits(nc, max_waits=1):
    """walrus codegen rejects instructions carrying more than `max_waits`
    sync waits; hoist the extras onto same-engine nops placed just before."""
    import bass_rust
    import concourse.mybir as mybir
    cnt = 0
    for bb in nc.main_func.blocks:
        insts = bb.instructions
        i = 0
        while i < len(insts):
            ins = insts[i]
            si = getattr(ins, 'sync_info', None)
            if si is not None and si.on_wait and len(si.on_wait) > max_waits:
                waits = list(si.on_wait)
                extras, keep = waits[:-max_waits], waits[-max_waits:]
                for w in extras:
                    nop = mybir.InstNoOp(name=f"I-swx{cnt}", ins=[], outs=[])
                    cnt += 1
                    nop.engine = ins.engine
                    nop.sync_info = bass_rust.SyncInfo(on_wait=[w],
                                                       on_update=[])
                    insts.insert(i, nop)
                    i += 1
                ins.sync_info = bass_rust.SyncInfo(
                    on_wait=keep, on_update=list(si.on_update))
            i += 1
    return cnt


_built = None


def _build():
    """Build the SPMD bass program once. Returns (nc, n_split_waits)."""
    global _built
    if _built is not None:
        return _built
    _install_tilefix()
    from contextlib import ExitStack
    import concourse.bass as bass
    import concourse.tile as tile
    from concourse import mybir

    dt = mybir.dt
    ddt = dt.float16           # qkv/S/AV matmul operands
    rdt = dt.float32r          # proj matmul operands (o^T, W_p)
    f32 = dt.float32

    nc = bass.Bass("TRN2", target_bir_lowering=False, debug=False,
                   num_devices=8)

    # DRAM I/O (per core)
    xt_d = nc.dram_tensor("xt", [D, N], ddt, kind="ExternalInput")
    x2t_d = nc.dram_tensor("x2t", [D, N], ddt, kind="ExternalInput")
    wqk_d = nc.dram_tensor("wqk", [D, 2 * D], ddt, kind="ExternalInput")
    wv_d = nc.dram_tensor("wv", [D, D], ddt, kind="ExternalInput")
    wp_d = nc.dram_tensor("wp", [D, D], ddt, kind="ExternalInput")
    bias_d = nc.dram_tensor("bias", [P, D], f32, kind="ExternalInput")
    ones_d = nc.dram_tensor("ones", [P, NQT * H, 1], ddt,
                            kind="ExternalInput")
    id12_d = nc.dram_tensor("id12", [H, H], f32, kind="ExternalInput")
    id128_d = nc.dram_tensor("id128", [P, P], ddt, kind="ExternalInput")
    out_d = nc.dram_tensor("out", [2, N, D], f32, kind="ExternalOutput")

    with tile.TileContext(nc) as tc, ExitStack() as top:
        # PSUM pools: S double-buffered (4 banks) + po (2) + filler (2)
        pp_s = top.enter_context(tc.tile_pool(name="ps_s", bufs=2,
                                              space="PSUM"))
        pp_o = top.enter_context(tc.tile_pool(name="ps_o", bufs=1,
                                              space="PSUM"))
        pp_f = top.enter_context(tc.tile_pool(name="ps_f", bufs=1,
                                              space="PSUM"))
        dram = top.enter_context(tc.tile_pool(name="dram", bufs=1,
                                              space="DRAM"))
        persist = top.enter_context(tc.tile_pool(name="persist", bufs=1))
        pool_pt = top.enter_context(tc.tile_pool(name="pt", bufs=4))
        pool_rb = top.enter_context(tc.tile_pool(name="rb", bufs=4))
        pool_res = top.enter_context(tc.tile_pool(name="res", bufs=2))
        pool_rt = top.enter_context(tc.tile_pool(name="rt", bufs=2))

        # persistent SBUF
        qT = persist.tile([P, NDT, N], ddt, tag="qT")
        kT = persist.tile([P, NDT, N], ddt, tag="kT")
        kT2 = persist.tile([P, NDT, N], ddt, tag="kT2")
        vaug = persist.tile([P, NQT, H * AUG], ddt, tag="vaug")
        vaug2 = persist.tile([P, NQT, H * AUG], ddt, tag="vaug2")
        wp_t = persist.tile([P, NDT, D], ddt, tag="wp")
        bias_t = persist.tile([P, D], f32, tag="bias")
        id12_t = persist.tile([H, H], f32, tag="id12")
        id128_t = persist.tile([P, P], ddt, tag="id128")
        ot = [persist.tile([P, NDT, N], ddt, tag=f"ot{b}", name=f"ot{b}")
              for b in range(2)]
        r_all = [persist.tile([H, N], f32, tag=f"r{b}", name=f"r{b}")
                 for b in range(2)]
        rinvsw = [persist.tile([P, NQT * H], ddt, tag=f"risw{b}",
                               name=f"risw{b}") for b in range(2)]
        rinv = [persist.tile([H, N], ddt, tag=f"rinv{b}", name=f"rinv{b}")
                for b in range(2)]
        rinv_dram = [dram.tile([H, N], ddt, tag=f"rid{b}", name=f"rid{b}")
                     for b in range(2)]
        ones128 = persist.tile([P, N], f32, tag="ones128")
        nc.vector.memset(ones128[:], 1.0)


        def load_ones(vaug_t):
            for t in range(NQT):
                nc.sync.dma_start(
                    out=vaug_t[:, t, :].rearrange("p (h e) -> p h e",
                                                  e=AUG)[:, :, HD:AUG],
                    in_=ones_d[:, t * H:(t + 1) * H, :])

        # ================= phase A: q, k, k2, v =========================
        pool_x = top.enter_context(tc.tile_pool(name="xa", bufs=1))
        pool_x2 = top.enter_context(tc.tile_pool(name="x2a", bufs=1))
        pool_wst = top.enter_context(tc.tile_pool(name="wst", bufs=4))
        pool_wv = top.enter_context(tc.tile_pool(name="wvp", bufs=1))

        xt_t = pool_x.tile([P, NDT, N], ddt, tag="xt")
        for i in range(NDT):
            nc.sync.dma_start(out=xt_t[:, i, :],
                              in_=xt_d[i * P:(i + 1) * P, :])
        x2t_t = pool_x2.tile([P, NDT, N], ddt, tag="x2t")
        wv_t = pool_wv.tile([P, NDT, D], ddt, tag="wv")

        def qT_form(o, wcol0, xt_s, dst, dst2):
            """one output-row-block o of W^T x (and W^T x2 if dst2)."""
            wt = pool_wst.tile([P, NDT, P], ddt, tag="wt")
            nc.sync.dma_start(
                out=wt,
                in_=wqk_d[:, wcol0 + o * P: wcol0 + (o + 1) * P].rearrange(
                    "(i p) o -> p i o", p=P))
            for xs, ds in ((xt_s, dst), (x2t_t, dst2)):
                if ds is None:
                    continue
                ps = pp_s.tile([P, N], f32, tag="S")
                for i in range(NDT):
                    for c in range(NC):
                        nc.tensor.matmul(
                            ps[:, c * 512:(c + 1) * 512],
                            wt[:, i, :],
                            xs[:, i, c * 512:(c + 1) * 512],
                            start=(i == 0), stop=(i == NDT - 1),
                            skip_group_check=True)
                nc.scalar.copy(ds[:, o, :], ps[:])

        def v_tile(t, xt_s, vaug_t):
            """one token-block t of v = x W_v^T, written into vaug."""
            ps = pp_f.tile([P, D], f32, tag="F")
            for i in range(NDT):
                for c0, cn in ((0, 512), (512, 256)):
                    nc.tensor.matmul(
                        ps[:, c0:c0 + cn],
                        xt_s[:, i, t * P:(t + 1) * P],
                        wv_t[:, i, c0:c0 + cn],
                        start=(i == 0), stop=(i == NDT - 1),
                        skip_group_check=True)
            src = ps[:, 0:D].rearrange("p (h e) -> p h e", e=HD)
            dstv = vaug_t[:, t, :].rearrange("p (h e) -> p h e",
                                             e=AUG)[:, :, 0:HD]
            nc.vector.tensor_copy(dstv, src)

        for o in range(NDT):
            qT_form(o, 0, xt_t, qT, None)            # q
        nc.sync.dma_start(out=x2t_t,
                          in_=x2t_d[:].rearrange("(i p) n -> p i n", p=P))
        nc.sync.dma_start(out=wv_t,
                          in_=wv_d[:].rearrange("(i p) d -> p i d", p=P))
        load_ones(vaug)
        nc.sync.dma_start(out=wp_t,
                          in_=wp_d[:].rearrange("(g p) d -> p g d", p=P))
        nc.sync.dma_start(out=bias_t, in_=bias_d[:])
        nc.sync.dma_start(out=id12_t, in_=id12_d[:])
        nc.sync.dma_start(out=id128_t, in_=id128_d[:])
        for o in range(NDT):
            qT_form(o, D, xt_t, kT, kT2)             # k and k2 (shared W)
        load_ones(vaug2)
        for t in range(NQT):
            v_tile(t, xt_t, vaug)                    # v

        # ================= r-chain helpers ==============================
        def r_chain_front(b):
            """transpose r [12,N] -> [128,96], batched reciprocal,
            transpose back, copy to SBUF, bounce to DRAM. Returns list of
            PE thunks (transposes) + trailing ops emitted inline."""
            tp = pp_f.tile([P, NQT * H], f32, tag="F")
            for c in range(NQT):
                nc.tensor.matmul(
                    tp[:, c * H:(c + 1) * H],
                    r_all[b][:, c * P:(c + 1) * P],
                    id12_t[:],
                    is_transpose=True, skip_group_check=True)
            with nc.allow_low_precision(reason='1/r fp16'):
                nc.vector.reciprocal(rinvsw[b][:], tp[:])
            bp = pp_f.tile([H, N], ddt, tag="F")
            for c in range(NQT):
                nc.tensor.matmul(
                    bp[:, c * P:(c + 1) * P],
                    rinvsw[b][:, c * H:(c + 1) * H],
                    id128_t[:],
                    is_transpose=True, skip_group_check=True)
            nc.vector.tensor_copy(rinv[b][:], bp[:])
            nc.sync.dma_start(out=rinv_dram[b][:], in_=rinv[b][:])

        def norm_mul(b, h):
            # both TT inputs must share a start partition (walrus
            # samePartitionsAll); slice rb at the same offset as ot
            g, row = h // 2, (h % 2) * HD
            rb = pool_rb.tile([P, N], ddt, tag="rb")
            nc.sync.dma_start(
                out=rb[row:row + HD, :],
                in_=rinv_dram[b][h, :].partition_broadcast(HD))
            nc.vector.tensor_mul(ot[b][row:row + HD, g, :],
                                 ot[b][row:row + HD, g, :],
                                 rb[row:row + HD, :])

        def proj_qi(b, qi, tail=False):
            pool = pp_s if (tail and qi % 2 == 1) else pp_f
            ps = pool.tile([P, D], f32, tag="S" if (tail and qi % 2 == 1)
                           else "F")
            for g in range(NDT):
                for c0, cn in ((0, 512), (512, 256)):
                    nc.tensor.matmul(
                        ps[:, c0:c0 + cn],
                        ot[b][:, g, qi * P:(qi + 1) * P],
                        wp_t[:, g, c0:c0 + cn],
                        start=(g == 0), stop=(g == NDT - 1),
                        skip_group_check=True)
            res = pool_res.tile([P, D], f32, tag="res")
            nc.vector.tensor_add(res[:], ps[:], bias_t[:])
            nc.sync.dma_start(out=out_d[b, qi * P:(qi + 1) * P, :],
                              in_=res[:])

        # ================= phase B: attention ===========================
        def attention(b, kT_t, vaug_t, fillers):
            """per-head S->exp->AV pipeline; fillers is a list of
            (priority-ordered) thunks emitted into PE slack, ~1/head."""
            for h in range(H):
                g, row = h // 2, (h % 2) * HD
                po = pp_o.tile([AUG, N], f32, tag="po")
                pts = []

                def emit_S_exp(kj):
                    ps = pp_s.tile([P, N], f32, tag="S")
                    for c in range(NC):
                        nc.tensor.matmul(
                            ps[:, c * 512:(c + 1) * 512],
                            kT_t[row:row + HD, g, kj * P:(kj + 1) * P],
                            qT[row:row + HD, g, c * 512:(c + 1) * 512],
                            start=True, stop=True, skip_group_check=True)
                    pt = pool_pt.tile([P, N], ddt, tag="pt")
                    nc.scalar.activation(
                        pt[:], ps[:],
                        mybir.ActivationFunctionType.Exp, scale=SCALE)
                    pts.append(pt)

                def emit_av(kj):
                    for c in range(NC):
                        nc.tensor.matmul(
                            po[:, c * 512:(c + 1) * 512],
                            vaug_t[:, kj, h * AUG:(h + 1) * AUG],
                            pts[kj][:, c * 512:(c + 1) * 512],
                            start=(kj == 0), stop=(kj == NQT - 1),
                            skip_group_check=True)

                emit_S_exp(0)
                emit_S_exp(1)
                for kj in range(2, NQT):
                    emit_S_exp(kj)
                    emit_av(kj - 2)
                    if kj == 4 and fillers:
                        fillers.pop(0)()
                emit_av(NQT - 2)
                emit_av(NQT - 1)
                # evict o^T (unnormalized) and the row-sum r; TT-mul by
                # ones (not copy): walrus allows partition-offset APs on
                # TENSOR_TENSOR in f32 but not on 16-bit/copy forms
                nc.vector.tensor_mul(ot[b][row:row + HD, g, :], po[0:HD, :],
                                     ones128[0:HD, :])
                rtmp = pool_rt.tile([1, N], f32, tag="rtmp")
                nc.vector.tensor_mul(rtmp[0:1, :], po[HD:HD + 1, :],
                                     ones128[HD:HD + 1, :])
                # engines cannot write partition h directly; DMA-scatter
                nc.sync.dma_start(out=r_all[b][h:h + 1, :], in_=rtmp[0:1, :])

        # B1: branch-1 attention + v2-formation fillers
        fillers1 = [(lambda t=t: v_tile(t, x2t_t, vaug2)) for t in range(NQT)]
        attention(0, kT, vaug, fillers1)
        while fillers1:
            fillers1.pop(0)()

        # B2: branch-2 attention + (r-chain b0, norm b0, proj b0) fillers
        fillers2 = [lambda: r_chain_front(0)]
        fillers2 += [(lambda h=h: (norm_mul(0, 2 * h), norm_mul(0, 2 * h + 1)))
                     for h in range(H // 2)]
        fillers2 += [(lambda qi=qi: proj_qi(0, qi)) for qi in range(NQT)]
        attention(1, kT2, vaug2, fillers2)
        while fillers2:
            fillers2.pop(0)()

        # tail: branch-2 r-chain, normalize, proj
        r_chain_front(1)
        for h in range(H):
            norm_mul(1, h)
        for qi in range(NQT):
            proj_qi(1, qi, tail=True)

    n = _split_multiwaits(nc)
    _built = (nc, n)
    return _built


def _host_prep(x, x2, qkv_w, proj_w, proj_b):
    """-> list of 8 per-core input maps. All matmul operands fp16."""
    f16 = lambda a: np.ascontiguousarray(a, dtype=np.float16)

    xt = np.transpose(np.asarray(x), (0, 2, 1))
    x2t = np.transpose(np.asarray(x2), (0, 2, 1))
    wqk = f16(np.asarray(qkv_w)[:2 * D].T)      # [768, 1536]
    wv = f16(np.asarray(qkv_w)[2 * D:].T)       # [768, 768]
    wp = f16(np.asarray(proj_w).T)              # [768, 768]
    bias = np.broadcast_to(np.asarray(proj_b, dtype=np.float32),
                           (P, D)).copy()
    ones = np.ones((P, NQT * H, 1), dtype=np.float16)
    id12 = np.eye(H, dtype=np.float32)
    id128 = np.eye(P, dtype=np.float16)
    maps = []
    for c in range(B):
        maps.append({
            "xt": f16(xt[c]), "x2t": f16(x2t[c]),
            "wqk": wqk, "wv": wv, "wp": wp, "bias": bias,
            "ones": ones, "id12": id12, "id128": id128,
        })
    return maps


def kernel(x, x2, qkv_w, proj_w, proj_b, trace=False, tmpdir=None):
    nc, _ = _build()
    from concourse.bass_utils import run_bass_kernel_spmd
    in_maps = _host_prep(x, x2, qkv_w, proj_w, proj_b)
    res = run_bass_kernel_spmd(nc, in_maps, list(range(B)), trace=trace,
                               tmpdir=tmpdir)
    kernel.last_exec_time_ns = res.exec_time_ns
    out = np.stack([res.results[c]["out"] for c in range(B)])  # [B,2,N,D]
    out1 = np.ascontiguousarray(out[:, 0])
    out2 = np.ascontiguousarray(out[:, 1])
    return (out1, out2)


kernel.last_exec_time_ns = None
